# revision 29
# baseline (speedup 1.0000x reference)
"""Multi-head causal attention (B=4, S=2048, D=2048, H=16) on 8 TRN2 NeuronCores.

Sharding: 2-D over (batch, head-group). Core c handles batch b = c//2 and head
group g = c%2 (8 of the 16 heads). Each core computes, for its (b, g):
  - V   = x_b @ Wv[:, g-cols]     (natural [s, hd] layout, per s-chunk)
  - Q^T, K^T                      ([hd, s] layout, per head)
  - P^T = exp(scale * scores^T) per (head, q-tile), causally masked at
    128-row granularity (fully-masked k-chunks skipped entirely)
  - ctx^T = V^T @ P^T accumulated over k-chunks; softmax denominators via
    DVE chunk-sums + a GpSimd partition all-reduce; 1/l normalization fused
    into the PSUM->SBUF copy
  - partial out = ctx @ Wo[g-rows, :]  ([s, d_out] fp32)
Host: out[b] = partial[2b] + partial[2b+1] + bo.

Compute dtype fp16 (TensorE full rate); matmul accumulation fp32 in PSUM.
"""

import math
from contextlib import ExitStack

import numpy as np

import concourse.bass as bass
import concourse.bass_isa as bass_isa
import concourse.mybir as mybir
import concourse.tile as tile
from concourse import tile as _tile_mod
from concourse.bass_utils import run_bass_kernel_spmd
from concourse.vector_clock import ScopedClock, VectorClock

# ---------------------------------------------------------------------------
# Workaround: this walrus build rejects instructions carrying more than one
# sem wait ("Too many sync wait commands"). Engines execute in order, so a
# wait hoisted onto an immediately-preceding nop on the same engine is
# semantically identical. Hook the two places Tile emits instructions with
# multi-wait sync_info: per-instruction commit, and the final drain.
# ---------------------------------------------------------------------------
_WAIT_LIMIT = 1
_N_PROCS = 64

_orig_add_instruction = _tile_mod.TileContext._add_instruction


def _add_instruction_split_waits(self, inst):
    si = inst.sync_info
    if (
        si is not None
        and si.on_wait
        and len(si.on_wait) > _WAIT_LIMIT
        and inst.engine != mybir.EngineType.Unassigned
    ):
        waits = list(si.on_wait)
        excess, keep = waits[:-_WAIT_LIMIT], waits[-_WAIT_LIMIT:]
        for i, w in enumerate(excess):
            nop = mybir.InstNoOp(name=f"{inst.name}-wsplit{i}", ins=[], outs=[])
            nop.engine = inst.engine
            nop.sync_info = mybir.SyncInfo(on_wait=[w], on_update=[])
            _orig_add_instruction(self, nop)
        si.on_wait = keep
    _orig_add_instruction(self, inst)


_tile_mod.TileContext._add_instruction = _add_instruction_split_waits


def _split_drain_and_barrier(self, tick_clock, wait_clock):
    gc = tick_clock.global_clock
    for p in range(_N_PROCS):
        try:
            cur = gc.peek_next(p) - 1
        except Exception:
            break
        if cur <= 0:
            continue
        v = VectorClock()
        v.require_at_least(p, cur)
        nop = self.nc.sync.nop(nofuse=True)
        wait_clock.add_sem_waits(nop.ins, ScopedClock({None: v}))
    self.nc.sync.drain()
    self.nc.all_engine_barrier()
    assert self.sems is not None
    popped = self.nc._tile_sem_poison_stack.pop()
    assert popped is self._sem_poison
    self.nc.clear_and_free_semaphores(list(self.sems.allocated().values()))
    self.nc.all_engine_barrier()


_tile_mod.TileContext._drain_and_barrier = _split_drain_and_barrier

# ---------------------------------------------------------------------------

B, S, D = 4, 2048, 2048
H, HD = 16, 128
G = 2                    # head groups == cores per batch
HPC = H // G             # heads per core
COLS = HPC * HD          # 1024 projection columns per core
P = 128
KD = D // P              # 16 contraction chunks over D
NJ = S // 512            # 4 q-tiles of 512 per head
SCALE = 1.0 / math.sqrt(HD)
F16 = mybir.dt.float16
F32 = mybir.dt.float32
EXP = mybir.ActivationFunctionType.Exp
ALU = mybir.AluOpType


def build_kernel(iters: int = 1, ablate: frozenset = frozenset()) -> bass.Bass:
    nc = bass.Bass()
    xt = nc.dram_tensor("xt", [D, S], F16, kind="ExternalInput")
    wq = nc.dram_tensor("wq", [D, COLS], F16, kind="ExternalInput")
    wk = nc.dram_tensor("wk", [D, COLS], F16, kind="ExternalInput")
    wv = nc.dram_tensor("wv", [D, COLS], F16, kind="ExternalInput")
    wo = nc.dram_tensor("wo", [COLS, D], F16, kind="ExternalInput")
    out = nc.dram_tensor("out", [S, D], F32, kind="ExternalOutput")

    with tile.TileContext(nc) as tc, ExitStack() as top:
        singles = top.enter_context(tc.tile_pool(name="singles", bufs=1))

        # Causal masking is folded into the scores matmul group: after the
        # K^T@Q matmul, a second accumulating matmul adds -6e4 at masked
        # positions ((-6e4*I) @ invmask), so exp underflows to exactly 0.
        # invmask[m][p, f] = 1 where MASKED, i.e. f < p + 128*m.
        invmasks = singles.tile([P, 4, 512], F16)
        nc.vector.memset(invmasks, 1.0)
        for m in range(4):
            # masked iff f < p + 128m  <=>  p - f + 128m - 1 >= 0
            nc.gpsimd.affine_select(
                out=invmasks[:, m, :],
                in_=invmasks[:, m, :],
                compare_op=ALU.is_ge,
                fill=0.0,
                base=128 * m - 1,
                pattern=[[-1, 512]],
                channel_multiplier=1,
            )
        negi = singles.tile([P, P], F16)
        nc.vector.memset(negi, -60000.0)
        nc.gpsimd.affine_select(
            out=negi,
            in_=negi,
            compare_op=ALU.is_equal,
            fill=0.0,
            base=0,
            pattern=[[-1, P]],
            channel_multiplier=1,
        )

        def body(_it):
            with ExitStack() as es:
                vp = es.enter_context(tc.tile_pool(name="vp", bufs=1))
                qtp = es.enter_context(tc.tile_pool(name="qtp", bufs=1))
                ktp = es.enter_context(tc.tile_pool(name="ktp", bufs=1))
                v_sb = vp.tile([P, KD, COLS], F16)    # V[s-chunk][:, head-cols]
                qt_sb = qtp.tile([P, HPC, S], F16)    # Q^T per head
                kt_sb = ktp.tile([P, HPC, S], F16)    # K^T per head

                # ================= Phase 1: projections =================
                with ExitStack() as p1:
                    wvp = p1.enter_context(tc.tile_pool(name="wvp", bufs=1))
                    wv_sb = wvp.tile([P, KD, COLS], F16)
                    nc.gpsimd.dma_start(
                        out=wv_sb, in_=wv[:, :].rearrange("(a p) m -> p a m", p=P)
                    )
                    xtp = p1.enter_context(tc.tile_pool(name="xtp", bufs=2))
                    wsp = p1.enter_context(tc.tile_pool(name="wsp", bufs=3))
                    psv = p1.enter_context(
                        tc.tile_pool(name="psv", bufs=2, space="PSUM")
                    )
                    psq = p1.enter_context(
                        tc.tile_pool(name="psq", bufs=4, space="PSUM")
                    )
                    # Stream x^T per quarter (512 s-positions) so compute
                    # starts after the first 2.1MB instead of 8.4MB.
                    for quarter in range(4):
                        xt_sb = xtp.tile([P, KD, 512], F16, tag="xth")
                        nc.sync.dma_start(
                            out=xt_sb,
                            in_=xt[:, quarter * 512:(quarter + 1) * 512]
                            .rearrange("(a p) s -> p a s", p=P),
                        )
                        # Q^T / K^T for this quarter's q-positions (j=quarter)
                        for wmat, dst in ((wq, qt_sb), (wk, kt_sb)):
                            for h in range(HPC):
                                ws = wsp.tile([P, KD, HD], F16, tag="ws")
                                nc.sync.dma_start(
                                    out=ws,
                                    in_=wmat[:, h * HD:(h + 1) * HD]
                                    .rearrange("(a p) m -> p a m", p=P),
                                )
                                ps = psq.tile([P, 512], F32, tag="psq")
                                for kd in range(KD):
                                    nc.tensor.matmul(
                                        ps,
                                        lhsT=ws[:, kd, :],
                                        rhs=xt_sb[:, kd, :],
                                        start=(kd == 0),
                                        stop=(kd == KD - 1),
                                    )
                                nc.scalar.copy(
                                    dst[:, h, quarter * 512:(quarter + 1) * 512], ps
                                )
                        # V for this quarter's s-chunks
                        for sil in range(4):
                            sg = quarter * 4 + sil
                            for nh in range(2):
                                ps = psv.tile([P, 512], F32, tag="psv")
                                for kd in range(KD):
                                    nc.tensor.matmul(
                                        ps,
                                        lhsT=xt_sb[:, kd, sil * 128:(sil + 1) * 128],
                                        rhs=wv_sb[:, kd, nh * 512:(nh + 1) * 512],
                                        start=(kd == 0),
                                        stop=(kd == KD - 1),
                                    )
                                nc.scalar.copy(
                                    v_sb[:, sg, nh * 512:(nh + 1) * 512], ps
                                )

                # ================= Phase 2: attention =================
                ctxp = es.enter_context(tc.tile_pool(name="ctxp", bufs=1))
                ctxt_sb = ctxp.tile([P, HPC, S], F16)  # ctx^T per head
                # Prefetch the output-projection weights during attention so
                # phase 3 doesn't stall on a 4.2MB DMA.
                wop = es.enter_context(tc.tile_pool(name="wop", bufs=1))
                wo_sb = wop.tile([P, HPC, D], F16)
                nc.gpsimd.dma_start(
                    out=wo_sb, in_=wo[:, :].rearrange("(a p) m -> p a m", p=P)
                )
                if "p2" in ablate:
                    return
                with ExitStack() as p2:
                    ptp = p2.enter_context(tc.tile_pool(name="ptp", bufs=7))
                    accp = p2.enter_context(tc.tile_pool(name="accp", bufs=4))
                    rsp = p2.enter_context(tc.tile_pool(name="rsp", bufs=4))
                    outp = p2.enter_context(tc.tile_pool(name="outp", bufs=2))
                    pss = p2.enter_context(
                        tc.tile_pool(name="pss", bufs=2, space="PSUM")
                    )
                    psc = p2.enter_context(
                        tc.tile_pool(name="psc", bufs=3, space="PSUM")
                    )
                    pso = p2.enter_context(
                        tc.tile_pool(name="pso", bufs=1, space="PSUM")
                    )
                    # j outer: a q-column (all heads) completes ctx^T for
                    # s-positions [512j, 512j+512), whose output projection is
                    # then interleaved — its matmuls fill PE idle while the
                    # next column's DVE softmax chain runs.
                    for j in range(NJ):
                        nch = 4 * (j + 1)  # causal: k-chunks 0..4j+3
                        for h in range(HPC):
                            ps_ctx = psc.tile([P, 512], F32, tag="psc")
                            acc = accp.tile([P, 512], F32, tag="acc")
                            # process k-chunks in pairs: one [128,1024] psum
                            # (2 banks) per pair so exp runs as a single wide
                            # activation (halves ACT per-op overhead).
                            for cp in range(nch // 2):
                                ps_s = pss.tile([P, 1024], F32, tag="pss")
                                for k in range(2):
                                    ci = 2 * cp + k
                                    m = ci - 4 * j
                                    nc.tensor.matmul(
                                        ps_s[:, k * 512:(k + 1) * 512],
                                        lhsT=kt_sb[:, h, ci * 128:(ci + 1) * 128],
                                        rhs=qt_sb[:, h, j * 512:(j + 1) * 512],
                                        start=True,
                                        stop=(k == 1 and (m < 0 or "mask" in ablate)),
                                        skip_group_check=True,
                                    )
                                    if m >= 0 and "mask" not in ablate:
                                        nc.tensor.matmul(
                                            ps_s[:, k * 512:(k + 1) * 512],
                                            lhsT=negi,
                                            rhs=invmasks[:, m, :],
                                            start=False,
                                            stop=(k == 1),
                                            skip_group_check=True,
                                        )
                                pt = ptp.tile([P, 1024], F16, tag="pt")
                                if "exp" in ablate:
                                    nc.scalar.copy(pt, ps_s)
                                else:
                                    nc.scalar.activation(pt, ps_s, EXP, scale=SCALE)
                                for k in range(2):
                                    ci = 2 * cp + k
                                    nc.tensor.matmul(
                                        ps_ctx,
                                        lhsT=v_sb[:, ci, h * HD:(h + 1) * HD],
                                        rhs=pt[:, k * 512:(k + 1) * 512],
                                        start=(ci == 0),
                                        stop=(ci == nch - 1),
                                    )
                                # accumulate exp chunk-sums R on DVE (fp32)
                                if "norm" in ablate:
                                    pass
                                elif cp == 0:
                                    nc.vector.scalar_tensor_tensor(
                                        out=acc,
                                        in0=pt[:, 0:512],
                                        scalar=1.0,
                                        in1=pt[:, 512:1024],
                                        op0=ALU.bypass,
                                        op1=ALU.add,
                                    )
                                else:
                                    for k in range(2):
                                        nc.vector.scalar_tensor_tensor(
                                            out=acc,
                                            in0=pt[:, k * 512:(k + 1) * 512],
                                            scalar=1.0,
                                            in1=acc,
                                            op0=ALU.bypass,
                                            op1=ALU.add,
                                        )
                            # l[q] = per-column sum of R; GpSimd all-reduce
                            # leaves every partition holding l, a full-lane
                            # reciprocal then gives the broadcast 1/l.
                            if "norm" in ablate:
                                nc.vector.tensor_copy(
                                    ctxt_sb[:, h, j * 512:(j + 1) * 512], ps_ctx
                                )
                            else:
                                rs = rsp.tile([P, 512], F32, tag="rs")
                                nc.gpsimd.partition_all_reduce(
                                    rs, acc, channels=P,
                                    reduce_op=bass_isa.ReduceOp.add,
                                )
                                nc.vector.reciprocal(rs, rs)
                                nc.vector.scalar_tensor_tensor(
                                    out=ctxt_sb[:, h, j * 512:(j + 1) * 512],
                                    in0=ps_ctx,
                                    scalar=1.0,
                                    in1=rs,
                                    op0=ALU.bypass,
                                    op1=ALU.mult,
                                )
                        # output projection for this column's s-chunks
                        for so in ([] if "p3" in ablate else range(4 * j, 4 * j + 4)):
                            for half in range(2):
                                osb = outp.tile([P, D // 2], F32, tag="osb")
                                for dh in range(2):
                                    do = half * 2 + dh
                                    ps = pso.tile([P, 512], F32, tag="pso")
                                    for kh in range(HPC):
                                        nc.tensor.matmul(
                                            ps,
                                            lhsT=ctxt_sb[:, kh, so * 128:(so + 1) * 128],
                                            rhs=wo_sb[:, kh, do * 512:(do + 1) * 512],
                                            start=(kh == 0),
                                            stop=(kh == HPC - 1),
                                        )
                                    if do % 2 == 0:
                                        nc.scalar.copy(
                                            osb[:, dh * 512:(dh + 1) * 512], ps
                                        )
                                    else:
                                        nc.vector.tensor_copy(
                                            osb[:, dh * 512:(dh + 1) * 512], ps
                                        )
                                nc.sync.dma_start(
                                    out=out[so * 128:(so + 1) * 128,
                                            half * (D // 2):(half + 1) * (D // 2)],
                                    in_=osb,
                                )

        if iters == 1:
            body(0)
        else:
            with tc.For_i(0, iters) as i:
                body(i)

    # populate .instr bytes for extended-inst InstISA subclasses
    # (partition_all_reduce) — without this walrus fails "ISA wrong length" —
    # and insert GPSIMD ucode library reloads so the Q7 cores actually have
    # the attn library (partition_all_reduce) resident when those run.
    from concourse.library_overlay import lower_extended_insts
    from concourse.library_config import all_libraries, standard
    import bass_rust as _bass_rust

    inst_type_to_lib_mask = {}
    for lib in all_libraries:
        for it in lib.instructions:
            inst_type_to_lib_mask[it] = inst_type_to_lib_mask.get(it, 0) | (
                1 << lib.index
            )
    _bass_rust.insert_library_loads(
        nc, inst_type_to_lib_mask, len(all_libraries), standard.index
    )
    lower_extended_insts(nc)
    return nc


def make_in_maps(x, Wq, Wk, Wv, Wo):
    """Host-side sharding: slice + transpose + fp16 cast per core."""
    x = np.asarray(x, dtype=np.float32)
    Wq = np.asarray(Wq, dtype=np.float32)
    Wk = np.asarray(Wk, dtype=np.float32)
    Wv = np.asarray(Wv, dtype=np.float32)
    Wo = np.asarray(Wo, dtype=np.float32)

    xts = [np.ascontiguousarray(x[b].T).astype(np.float16) for b in range(B)]
    wqg = [np.ascontiguousarray(Wq[:, g * COLS:(g + 1) * COLS]).astype(np.float16) for g in range(G)]
    wkg = [np.ascontiguousarray(Wk[:, g * COLS:(g + 1) * COLS]).astype(np.float16) for g in range(G)]
    wvg = [np.ascontiguousarray(Wv[:, g * COLS:(g + 1) * COLS]).astype(np.float16) for g in range(G)]
    wog = [np.ascontiguousarray(Wo[g * COLS:(g + 1) * COLS, :]).astype(np.float16) for g in range(G)]

    in_maps = []
    for c in range(8):
        b, g = divmod(c, 2)
        in_maps.append(
            {"xt": xts[b], "wq": wqg[g], "wk": wkg[g], "wv": wvg[g], "wo": wog[g]}
        )
    return in_maps


def assemble_output(results, bo):
    bo = np.asarray(bo, dtype=np.float32)
    out = np.empty((B, S, D), dtype=np.float32)
    for b in range(B):
        out[b] = results[2 * b]["out"] + results[2 * b + 1]["out"] + bo[None, :]
    return out


def kernel(x, Wq, Wk, Wv, Wo, bo):
    nc = build_kernel(iters=1)
    in_maps = make_in_maps(x, Wq, Wk, Wv, Wo)
    res = run_bass_kernel_spmd(nc, in_maps, core_ids=list(range(8)))
    return assemble_output(res.results, bo)
